# revision 1
# baseline (speedup 1.0000x reference)
"""GroupedQueryAttention forward on 8 Trainium2 NeuronCores (Bass/Tile).

Sharding (per spec hint): data-parallel over batch (B=2) x tensor-parallel
over KV-head groups (4 groups of 2 KV heads + their 8 query heads each).
Core c -> (batch b = c // 4, group g = c % 4).

Each core computes, for its batch element and its 8 query heads:
  qT/kT projections in transposed layout (lhsT = W, rhs = xT), V natural via
  on-chip PE transpose of vT; causal softmax without max-subtraction (scores
  are ~N(0,1) after the 1/sqrt(hd) scale, exp cannot overflow); the softmax
  denominator is produced by the same matmul as attn@V via a ones-column
  appended to V; normalization is folded into the o_proj stationary tiles.
  o_proj is row-parallel: each core emits a full [N, D] fp32 partial, and the
  host sums the 4 partials per batch element (the "all-reduce" of the o_proj).

All device compute is bf16 with fp32 PSUM accumulation. The host pre-casts
and pre-transposes x (xT) and pre-slices/reorders the weight shards so the
device performs no layout work on the inputs.
"""

import numpy as np

import concourse.bass as bass  # noqa: F401  (import keeps engine registry warm)
import concourse.mybir as mybir
import concourse.tile as tile
from concourse import bacc, bass_utils

# Problem shape (hardcoded per contract).
B, N, D = 2, 2048, 2048
NUM_HEADS = 32
NUM_KV_HEADS = 8
HD = 64                      # head dim
G = NUM_HEADS // NUM_KV_HEADS  # 4 query heads per kv head
N_CORES = 8
LQ = 8                       # local query heads per core (2 kv heads * G)
NT = D // 128                # 16 contraction tiles
NCHUNK = 4                   # token chunks of 512
CH = 512

_CACHE = {}


def _build():
    nc = bacc.Bacc("TRN2", target_bir_lowering=False, debug=False,
                   num_devices=N_CORES)
    f32, bf16 = mybir.dt.float32, mybir.dt.bfloat16

    xT = nc.dram_tensor("xT", [D, N], bf16, kind="ExternalInput")
    wq = nc.dram_tensor("wq", [D, 512], bf16, kind="ExternalInput")
    wk = nc.dram_tensor("wk", [D, 128], bf16, kind="ExternalInput")
    wv = nc.dram_tensor("wv", [D, 128], bf16, kind="ExternalInput")
    wo = nc.dram_tensor("wo", [512, D], bf16, kind="ExternalInput")
    msk = nc.dram_tensor("msk", [128, 4 * CH], bf16, kind="ExternalInput")
    iden = nc.dram_tensor("iden", [128, 128], bf16, kind="ExternalInput")
    sel = nc.dram_tensor("sel", [8, 4 * 128], f32, kind="ExternalInput")
    part = nc.dram_tensor("part", [N, D], f32, kind="ExternalOutput")

    with tile.TileContext(nc) as tc:
        with (
            tc.tile_pool(name="const", bufs=1) as cpool,
            tc.tile_pool(name="proj", bufs=1) as ppool,
            tc.tile_pool(name="work", bufs=4) as wpool,
            tc.tile_pool(name="att", bufs=1) as apool,
            tc.tile_pool(name="stage", bufs=3) as spool,
            tc.tile_pool(name="ps_s", bufs=2, space="PSUM") as ps_s,
            tc.tile_pool(name="ps_av", bufs=4, space="PSUM") as ps_av,
            tc.tile_pool(name="ps_m", bufs=1, space="PSUM") as ps_m,
        ):
            # ---- load constants / inputs to SBUF -------------------------
            xt = cpool.tile([128, NT * N], bf16, tag="xt")
            nc.sync.dma_start(
                xt[:].rearrange("p (t n) -> p t n", t=NT),
                xT.ap().rearrange("(t p) n -> p t n", p=128))
            wq_t = cpool.tile([128, NT * 512], bf16, tag="wq")
            nc.sync.dma_start(
                wq_t[:].rearrange("p (t o) -> p t o", t=NT),
                wq.ap().rearrange("(t p) o -> p t o", p=128))
            wk_t = cpool.tile([128, NT * 128], bf16, tag="wk")
            nc.sync.dma_start(
                wk_t[:].rearrange("p (t o) -> p t o", t=NT),
                wk.ap().rearrange("(t p) o -> p t o", p=128))
            wv_t = cpool.tile([128, NT * 128], bf16, tag="wv")
            nc.sync.dma_start(
                wv_t[:].rearrange("p (t o) -> p t o", t=NT),
                wv.ap().rearrange("(t p) o -> p t o", p=128))
            wo_t = cpool.tile([128, 4 * D], bf16, tag="wo")
            nc.sync.dma_start(
                wo_t[:].rearrange("p (t o) -> p t o", t=4),
                wo.ap().rearrange("(t p) o -> p t o", p=128))
            msk_t = cpool.tile([128, 4 * CH], bf16, tag="msk")
            nc.sync.dma_start(msk_t[:], msk.ap()[:])
            id_t = cpool.tile([128, 128], bf16, tag="iden")
            nc.sync.dma_start(id_t[:], iden.ap()[:])
            ones64 = cpool.tile([1, 64], f32, tag="ones64")
            nc.vector.memset(ones64[:], 1.0)
            sel_t = cpool.tile([8, 4 * 128], f32, tag="sel")
            nc.sync.dma_start(sel_t[:], sel.ap()[:])

            # ---- projections --------------------------------------------
            # kT2 [128 (2 kv heads x 64), N]
            kt2 = ppool.tile([128, N], bf16, tag="kt2")
            for j in range(N // CH):
                ps = ps_m.tile([128, CH], f32, tag="misc")
                for t in range(NT):
                    nc.tensor.matmul(
                        ps[:], wk_t[:, t * 128:(t + 1) * 128],
                        xt[:, t * N + j * CH: t * N + (j + 1) * CH],
                        start=(t == 0), stop=(t == NT - 1))
                nc.scalar.activation(kt2[:, j * CH:(j + 1) * CH], ps[:],
                                     mybir.ActivationFunctionType.Copy)
            # vT [128, N] then transpose to V3 [128, 16*130] (V + ones col)
            v3 = apool.tile([128, 16 * 130], bf16, tag="v3")
            nc.vector.memset(v3[:], 1.0)
            for j in range(N // CH):
                ps = ps_m.tile([128, CH], f32, tag="misc")
                for t in range(NT):
                    nc.tensor.matmul(
                        ps[:], wv_t[:, t * 128:(t + 1) * 128],
                        xt[:, t * N + j * CH: t * N + (j + 1) * CH],
                        start=(t == 0), stop=(t == NT - 1))
                vt_s = spool.tile([128, CH], bf16, tag="vt")
                nc.scalar.activation(vt_s[:], ps[:],
                                     mybir.ActivationFunctionType.Copy)
                for s in range(4):       # 4 m-tiles of 128 in this chunk
                    mt = 4 * j + s
                    pst = ps_m.tile([128, 128], bf16, tag="tr")
                    nc.tensor.transpose(pst[:], vt_s[:, s * 128:(s + 1) * 128],
                                        id_t[:])
                    nc.vector.tensor_copy(v3[:, mt * 130: mt * 130 + 64],
                                          pst[:, 0:64])
                    nc.vector.tensor_copy(v3[:, mt * 130 + 65: mt * 130 + 129],
                                          pst[:, 64:128])
            # qT2 chunks a=0..3: [128 (head a | head a+4), N]
            qt2 = []
            for a in range(4):
                qa = ppool.tile([128, N], bf16, tag=f"qt2_{a}")
                for j in range(N // CH):
                    ps = ps_m.tile([128, CH], f32, tag="misc")
                    for t in range(NT):
                        nc.tensor.matmul(
                            ps[:], wq_t[:, t * 512 + a * 128: t * 512 + (a + 1) * 128],
                            xt[:, t * N + j * CH: t * N + (j + 1) * CH],
                            start=(t == 0), stop=(t == NT - 1))
                    nc.scalar.activation(qa[:, j * CH:(j + 1) * CH], ps[:],
                                         mybir.ActivationFunctionType.Copy)
                qt2.append(qa)

            # ---- attention + o_proj per token chunk ---------------------
            for ci in range(NCHUNK):
                n0 = ci * CH
                mt_hi = 4 * ci + 4          # m-tiles 0..mt_hi-1
                aot = []                     # attn_outT tiles per pair
                sum8 = apool.tile([1, 8 * CH], f32, tag="sum8")
                for wave in range(2):
                    for a in (2 * wave, 2 * wave + 1):
                        pa0 = ps_av.tile([128, CH], f32, tag="av")
                        pa1 = ps_av.tile([128, CH], f32, tag="av")
                        for mt in range(mt_hi):
                            diag = mt - 4 * ci
                            ss0 = ps_s.tile([128, CH], f32, tag="s")
                            ss1 = ps_s.tile([128, CH], f32, tag="s")
                            nc.tensor.matmul(
                                ss0[:], kt2[0:64, mt * 128:(mt + 1) * 128],
                                qt2[a][0:64, n0:n0 + CH],
                                start=True, stop=True)
                            nc.tensor.matmul(
                                ss1[:], kt2[64:128, mt * 128:(mt + 1) * 128],
                                qt2[a][64:128, n0:n0 + CH],
                                start=True, stop=True)
                            pt0 = wpool.tile([128, CH], bf16, tag="pt")
                            pt1 = wpool.tile([128, CH], bf16, tag="pt")
                            nc.scalar.activation(
                                pt0[:], ss0[:],
                                mybir.ActivationFunctionType.Exp, scale=0.125)
                            nc.scalar.activation(
                                pt1[:], ss1[:],
                                mybir.ActivationFunctionType.Exp, scale=0.125)
                            if diag >= 0:
                                mslc = msk_t[:, diag * CH:(diag + 1) * CH]
                                nc.vector.tensor_mul(pt0[:], pt0[:], mslc)
                                nc.vector.tensor_mul(pt1[:], pt1[:], mslc)
                            nc.tensor.matmul(
                                pa0[0:65, :], v3[:, mt * 130: mt * 130 + 65],
                                pt0[:], start=(mt == 0), stop=(mt == mt_hi - 1))
                            nc.tensor.matmul(
                                pa1[0:65, :], v3[:, mt * 130 + 65: mt * 130 + 130],
                                pt1[:], start=(mt == 0), stop=(mt == mt_hi - 1))
                        ao = apool.tile([128, CH], bf16, tag=f"ao_{a}")
                        nc.vector.tensor_copy(ao[0:64, :], pa0[0:64, :])
                        nc.vector.tensor_copy(ao[64:128, :], pa1[0:64, :])
                        nc.vector.tensor_copy(sum8[0:1, a * CH:(a + 1) * CH], pa0[64:65, :])
                        nc.vector.tensor_copy(sum8[0:1, (a + 4) * CH:(a + 5) * CH], pa1[64:65, :])
                        aot.append(ao)
                aos = []
                for a in range(4):
                    rb = ps_m.tile([128, CH], f32, tag="misc")
                    nc.tensor.matmul(rb[0:64, :], ones64[0:1, :],
                                     sum8[0:1, a * CH:(a + 1) * CH],
                                     start=True, stop=True, tile_position=(0, 0))
                    nc.tensor.matmul(rb[64:128, :], ones64[0:1, :],
                                     sum8[0:1, (a + 4) * CH:(a + 5) * CH],
                                     start=True, stop=True, tile_position=(0, 64))
                    rbr = spool.tile([128, CH], f32, tag="rbr")
                    nc.vector.reciprocal(rbr[:], rb[:])
                    an = apool.tile([128, CH], bf16, tag=f"aos_{a}")
                    nc.vector.tensor_mul(an[:], aot[a][:], rbr[:])
                    aos.append(an)
                # o_proj: out[n, :] += sum_c attn_outT_s[c, n] * Wo[c, :]
                for nt in range(4):
                    for dc in range(4):
                        po = ps_m.tile([128, CH], f32, tag="misc")
                        for a in range(4):
                            nc.tensor.matmul(
                                po[:], aos[a][:, nt * 128:(nt + 1) * 128],
                                wo_t[:, a * D + dc * CH: a * D + (dc + 1) * CH],
                                start=(a == 0), stop=(a == 3))
                        st = spool.tile([128, CH], f32, tag="ost")
                        nc.vector.tensor_copy(st[:], po[:])
                        nc.sync.dma_start(
                            part.ap()[n0 + nt * 128: n0 + (nt + 1) * 128,
                                      dc * CH:(dc + 1) * CH],
                            st[:])
    nc.compile()
    return nc


def _prep_in_maps(x, Wq, Wk, Wv, Wo):
    import jax.numpy as jnp

    def to_bf16(a):
        return np.asarray(jnp.asarray(np.asarray(a), dtype=jnp.bfloat16))

    # causal mask tiles for diagonal offsets 0..3 (within a 512 chunk)
    msk = np.zeros((128, 4 * CH), np.float32)
    for k in range(4):
        i = np.arange(128)[:, None]
        j = np.arange(CH)[None, :]
        msk[:, k * CH:(k + 1) * CH] = (i + 128 * k <= j).astype(np.float32)
    iden = np.eye(128, dtype=np.float32)
    sel = np.zeros((8, 4 * 128), np.float32)
    for a in range(4):
        sel[a, a * 128: a * 128 + 64] = 1.0
        sel[a + 4, a * 128 + 64: (a + 1) * 128] = 1.0

    in_maps = []
    for c in range(N_CORES):
        b, g = c // 4, c % 4
        qh = [8 * g + a for a in range(8)]      # global q heads for this core
        # Wq columns reordered into pair chunks [head a | head a+4]
        wq_cols = []
        for a in range(4):
            wq_cols.append(np.arange(qh[a] * HD, (qh[a] + 1) * HD))
            wq_cols.append(np.arange(qh[a + 4] * HD, (qh[a + 4] + 1) * HD))
        wq_r = np.asarray(Wq)[:, np.concatenate(wq_cols)]
        wo_rows = wq_cols  # same ordering for Wo rows
        wo_r = np.asarray(Wo)[np.concatenate(wo_rows), :]
        wk_s = np.asarray(Wk)[:, 2 * g * HD: (2 * g + 2) * HD]
        wv_s = np.asarray(Wv)[:, 2 * g * HD: (2 * g + 2) * HD]
        in_maps.append({
            "xT": to_bf16(np.asarray(x)[b].T),
            "wq": to_bf16(wq_r),
            "wk": to_bf16(wk_s),
            "wv": to_bf16(wv_s),
            "wo": to_bf16(wo_r),
            "msk": to_bf16(msk),
            "iden": to_bf16(iden),
            "sel": sel,
        })
    return in_maps


def kernel(x, Wq, Wk, Wv, Wo, trace=False):
    if "nc" not in _CACHE:
        _CACHE["nc"] = _build()
    nc = _CACHE["nc"]
    in_maps = _prep_in_maps(x, Wq, Wk, Wv, Wo)
    res = bass_utils.run_bass_kernel_spmd(
        nc, in_maps, core_ids=list(range(N_CORES)), trace=trace)
    _CACHE["last_result"] = res
    out = np.zeros((B, N, D), np.float32)
    for c in range(N_CORES):
        out[c // 4] += res.results[c]["part"]
    return out



# revision 11
# speedup vs baseline: 1.1419x; 1.1419x over previous
"""GroupedQueryAttention forward on 8 Trainium2 NeuronCores (Bass/Tile), v2.

Sharding (per spec hint): data-parallel over batch (B=2) x tensor-parallel
over KV-head groups (4 groups of 2 KV heads + their 8 query heads each).
Core c -> (batch b = c // 4, group g = c % 4).

v2 redesign vs baseline (573us):
  - scores row-tiled 2x: the two K=64 (per-kv-head) score matmuls run
    concurrently in disjoint PE row groups (tile_position (0,0)/(64,0)).
  - attn@V col-tiled 2x: the two M=64 AV matmuls run concurrently in
    disjoint PE col groups ((0,0)/(0,64)), both kv heads in one PSUM bank.
  - softmax denominators via 4 concurrent M=1 matmuls (ones stationary)
    col-tiled into one PSUM bank; reciprocal via DVE approx; broadcast to
    128 partitions via stride-0-source DMA; normalize fused into the PSUM
    evacuation tensor_tensor.
  - causal triangle trimming: diagonal-block matmuls/exps only cover the
    columns right of the diagonal (query >= key block start).
  - exp batched to FD=2*F per ACTIVATE from bf16 score PSUM banks.
  - mt-outer loop reuses kT/V stationaries across the 2 head pairs of a
    group; V transposed via DMA xbar instead of the PE.
  - software-pipelined emission: projection and o_proj matmul groups are
    interleaved into the attention loop as PE fillers so the tensor engine
    never idles while the scalar engine chews exps (keeps HAM at K=8/8).
  - o_proj partials written as bf16 (halves output DMA); host sums fp32.

All device compute bf16 with fp32 PSUM accumulation (bf16 PSUM for raw
scores only, pre-exp). Host pre-casts/pre-transposes x and pre-slices and
pair-reorders the weight shards.
"""

from collections import deque

import numpy as np

import concourse.bass as bass  # noqa: F401  (import keeps engine registry warm)
import concourse.mybir as mybir
import concourse.tile as tile
from concourse import bacc, bass_utils

# Problem shape (hardcoded per contract).
B, N, D = 2, 2048, 2048
NUM_HEADS = 32
NUM_KV_HEADS = 8
HD = 64
G = NUM_HEADS // NUM_KV_HEADS  # 4
N_CORES = 8
NT = D // 128                  # 16 contraction tiles
NCHUNK = 4                     # token chunks of 512
CH = 512

_CACHE = {}


def _build():
    nc = bacc.Bacc("TRN2", target_bir_lowering=False, debug=False,
                   num_devices=N_CORES)
    f32, bf16 = mybir.dt.float32, mybir.dt.bfloat16
    Exp = mybir.ActivationFunctionType.Exp

    xT = nc.dram_tensor("xT", [D, N], bf16, kind="ExternalInput")
    wq = nc.dram_tensor("wq", [D, 512], bf16, kind="ExternalInput")
    wk = nc.dram_tensor("wk", [D, 128], bf16, kind="ExternalInput")
    wv = nc.dram_tensor("wv", [D, 128], bf16, kind="ExternalInput")
    wo = nc.dram_tensor("wo", [512, D], bf16, kind="ExternalInput")
    msk = nc.dram_tensor("msk", [128, 128], bf16, kind="ExternalInput")
    part = nc.dram_tensor("part", [N, D], bf16, kind="ExternalOutput")

    with tile.TileContext(nc) as tc:
        with (
            tc.tile_pool(name="const", bufs=1) as cpool,
            tc.tile_pool(name="proj", bufs=1) as kpool,
            tc.tile_pool(name="work", bufs=2) as wpool,
            tc.tile_pool(name="ps_s", bufs=1, space="PSUM") as ps_s,
            tc.tile_pool(name="ps_pa", bufs=1, space="PSUM") as ps_pa,
            tc.tile_pool(name="ps_d", bufs=1, space="PSUM") as ps_d,
            tc.tile_pool(name="ps_proj", bufs=1, space="PSUM") as ps_proj,
        ):
            # ---- inputs / constants -------------------------------------
            wk_t = cpool.tile([128, NT * 128], bf16, tag="wk")
            nc.sync.dma_start(
                wk_t[:].rearrange("p (t o) -> p t o", t=NT),
                wk.ap().rearrange("(t p) o -> p t o", p=128))
            wv_t = cpool.tile([128, NT * 128], bf16, tag="wv")
            nc.sync.dma_start(
                wv_t[:].rearrange("p (t o) -> p t o", t=NT),
                wv.ap().rearrange("(t p) o -> p t o", p=128))
            xt = []
            for j in range(NCHUNK):
                xj = cpool.tile([128, NT * CH], bf16, tag=f"xt{j}")
                nc.sync.dma_start(
                    xj[:].rearrange("p (t n) -> p t n", t=NT),
                    xT.ap().rearrange("(t p) n -> p t n", p=128)
                    [:, :, j * CH:(j + 1) * CH])
                xt.append(xj)
            wq_t = cpool.tile([128, NT * 512], bf16, tag="wq")
            nc.sync.dma_start(
                wq_t[:].rearrange("p (t o) -> p t o", t=NT),
                wq.ap().rearrange("(t p) o -> p t o", p=128))
            wo_t = cpool.tile([128, 4 * D], bf16, tag="wo")
            nc.sync.dma_start(
                wo_t[:].rearrange("p (t o) -> p t o", t=4),
                wo.ap().rearrange("(t p) o -> p t o", p=128))
            msk_t = cpool.tile([128, 128], bf16, tag="msk")
            nc.sync.dma_start(msk_t[:], msk.ap()[:])
            ones1 = cpool.tile([128, 1], bf16, tag="ones1")
            nc.vector.memset(ones1[:], 1.0)
            # pre-warm the exp activation table while the DMAs run
            scr = cpool.tile([1, 8], f32, tag="scr")
            nc.vector.memset(scr[:], 0.0)
            nc.scalar.activation(scr[0:1, :], scr[0:1, :], Exp)

            # persistent projection outputs
            kt = [kpool.tile([128, CH], bf16, tag=f"kt{j}", name=f"kt{j}")
                  for j in range(NCHUNK)]
            v3 = [kpool.tile([128, 128], bf16, tag=f"v3_{m}", name=f"v3_{m}")
                  for m in range(N // 128)]
            qt = [[kpool.tile([128, CH], bf16, tag=f"qt{a}_{j}",
                              name=f"qt{a}_{j}")
                   for j in range(NCHUNK)] for a in range(4)]

            # ---- filler machinery ---------------------------------------
            fillers = deque()  # (emit_fn, est_pe_ns)

            def drain(budget_ns):
                spent = 0
                while fillers and spent < budget_ns:
                    fn, est = fillers.popleft()
                    fn()
                    spent += est

            def drain_all():
                while fillers:
                    fn, _ = fillers.popleft()
                    fn()

            # ---- projection thunks --------------------------------------
            def proj_chain(dst_evac, lhsT_col, j, n_chunks=4):
                """Returns thunk fns for one 16-deep contraction chain.

                lhsT_col(t) -> AP for the stationary tile;
                dst_evac(ps) emits the evacuation."""
                ps_box = {}

                def quarter(q):
                    def emit():
                        if q == 0:
                            ps_box["ps"] = ps_proj.tile(
                                [128, CH], f32, tag="proj", name="pps")
                        ps = ps_box["ps"]
                        for t in range(4 * q, 4 * q + 4):
                            nc.tensor.matmul(
                                ps[:], lhsT_col(t),
                                xt[j][:, t * CH:(t + 1) * CH],
                                start=(t == 0), stop=(t == NT - 1))
                        if q == 3:
                            dst_evac(ps)
                    return emit
                return [(quarter(q), 1400) for q in range(4)]

            def proj_thunks(j):
                th = []

                def kev(ps):
                    nc.vector.tensor_copy(kt[j][:], ps[:])
                th += proj_chain(kev, lambda t: wk_t[:, t * 128:(t + 1) * 128], j)

                def vev(ps):
                    vt_s = wpool.tile([128, CH], bf16, tag="vt", name="vt_s")
                    nc.vector.tensor_copy(vt_s[:], ps[:])
                    for s in range(4):
                        nc.sync.dma_start_transpose(
                            v3[4 * j + s][:], vt_s[:, s * 128:(s + 1) * 128])
                th += proj_chain(vev, lambda t: wv_t[:, t * 128:(t + 1) * 128], j)
                for a in range(4):
                    def qev(ps, a=a):
                        nc.vector.tensor_copy(qt[a][j][:], ps[:])
                    th += proj_chain(
                        qev,
                        lambda t, a=a: wq_t[:, t * 512 + a * 128:
                                            t * 512 + (a + 1) * 128], j)
                return th

            # ---- o_proj thunks ------------------------------------------
            def oproj_thunks(ci, an_tiles):
                n0 = ci * CH
                th = []
                for nt_ in range(4):
                    for dc in range(4):
                        def emit(nt_=nt_, dc=dc):
                            po = ps_proj.tile([128, CH], f32, tag="proj",
                                              name="po")
                            for a in range(4):
                                nc.tensor.matmul(
                                    po[:],
                                    an_tiles[a][:, nt_ * 128:(nt_ + 1) * 128],
                                    wo_t[:, a * D + dc * CH:
                                         a * D + (dc + 1) * CH],
                                    start=(a == 0), stop=(a == 3))
                            st = wpool.tile([128, CH], bf16, tag="st",
                                            bufs=3, name="st")
                            if (nt_ + dc) % 2 == 0:
                                nc.scalar.copy(st[:], po[:])
                            else:
                                nc.vector.tensor_copy(st[:], po[:])
                            nc.gpsimd.dma_start(
                                part.ap()[n0 + nt_ * 128:n0 + (nt_ + 1) * 128,
                                          dc * CH:(dc + 1) * CH],
                                st[:])
                        th.append((emit, 1400))
                return th

            # ---- attention ----------------------------------------------
            def attn_chunk(ci):
                n0 = ci * CH
                M = 4 * ci + 4
                an_tiles = []
                for g in range(2):
                    a0, a1 = 2 * g, 2 * g + 1
                    paA = ps_pa.tile([128, CH], f32, tag="paA", name="paA")
                    paB = ps_pa.tile([128, CH], f32, tag="paB", name="paB")
                    dn = ps_d.tile([128, CH], f32, tag="dn", name="dn")
                    pts = {}

                    def scores_exp(mt):
                        # score layout per head pair a: kv0 block at cols
                        # [0:F], kv1 block at cols [512:512+F] (bank 2);
                        # one FD=512+F exp covers both (cols [F:512] are
                        # junk for diagonal tiles and never read).
                        jmt, cmt = mt // 4, mt % 4
                        flo = max(0, (mt - 4 * ci) * 128)
                        F = CH - flo
                        pt_pair = []
                        for i, a in enumerate((a0, a1)):
                            ss = ps_s.tile([128, 1024], f32, tag=f"s{i}",
                                           name="ss")
                            nc.tensor.matmul(
                                ss[:, 0:F],
                                kt[jmt][0:64, cmt * 128:(cmt + 1) * 128],
                                qt[a][ci][0:64, flo:CH],
                                start=True, stop=True, tile_position=(0, 0))
                            nc.tensor.matmul(
                                ss[:, CH:CH + F],
                                kt[jmt][64:128, cmt * 128:(cmt + 1) * 128],
                                qt[a][ci][64:128, flo:CH],
                                start=True, stop=True, tile_position=(64, 0))
                            pt_ = wpool.tile([128, 1024], bf16, tag=f"pt{i}",
                                             name="pt")
                            nc.scalar.activation(pt_[:, 0:CH + F],
                                                 ss[:, 0:CH + F],
                                                 Exp, scale=0.125)
                            if mt >= 4 * ci:  # diagonal block: causal mask
                                nc.vector.tensor_mul(
                                    pt_[:, 0:128], pt_[:, 0:128], msk_t[:])
                                nc.vector.tensor_mul(
                                    pt_[:, CH:CH + 128], pt_[:, CH:CH + 128],
                                    msk_t[:])
                            pt_pair.append(pt_)
                        pts[mt] = (pt_pair, flo, F)

                    def av_denom(mt):
                        pt_pair, flo, F = pts.pop(mt)
                        first, last = (mt == 0), (mt == M - 1)
                        for i in range(2):
                            pa = paA if i == 0 else paB
                            pt_ = pt_pair[i]
                            nc.tensor.matmul(
                                pa[0:64, flo:CH], v3[mt][:, 0:64],
                                pt_[:, 0:F],
                                start=first, stop=last, tile_position=(0, 0))
                            nc.tensor.matmul(
                                pa[64:128, flo:CH], v3[mt][:, 64:128],
                                pt_[:, CH:CH + F],
                                start=first, stop=last, tile_position=(0, 64))
                        for pos, src in ((0, pt_pair[0][:, 0:F]),
                                         (32, pt_pair[0][:, CH:CH + F]),
                                         (64, pt_pair[1][:, 0:F]),
                                         (96, pt_pair[1][:, CH:CH + F])):
                            nc.tensor.matmul(
                                dn[pos:pos + 1, flo:CH], ones1[:], src,
                                start=first, stop=last,
                                tile_position=(0, pos))

                    for mt in range(M):
                        scores_exp(mt)
                        if mt > 0:
                            av_denom(mt - 1)
                        flo = max(0, (mt - 4 * ci) * 128)
                        F = CH - flo
                        slack = 2 * (2 * F + 352) / 1.2 - (5 * F / 2.4 + 400)
                        drain(max(0, slack))
                    av_denom(M - 1)

                    # normalize: 1/denom broadcast, fused into evacuation
                    d4r = wpool.tile([128, CH], f32, tag="d4r", name="d4r")
                    nc.vector.reciprocal_approx_fast(d4r[0:97, :], dn[0:97, :])
                    for i, a in enumerate((a0, a1)):
                        rbr = wpool.tile([128, CH], f32, tag=f"rbr{i}",
                                         name="rbr")
                        r0, r1 = 64 * i, 64 * i + 32
                        nc.sync.dma_start(
                            rbr[0:64, :],
                            d4r[r0:r0 + 1, :].unsqueeze(1)
                            .broadcast_to([1, 64, CH]))
                        nc.sync.dma_start(
                            rbr[64:128, :],
                            d4r[r1:r1 + 1, :].unsqueeze(1)
                            .broadcast_to([1, 64, CH]))
                        an = wpool.tile([128, CH], bf16, tag=f"an{a}",
                                        name=f"an{a}")
                        pa = paA if i == 0 else paB
                        nc.vector.tensor_mul(an[:], pa[:], rbr[:])
                        an_tiles.append(an)
                return an_tiles

            # ---- main schedule ------------------------------------------
            for fn, _ in proj_thunks(0):
                fn()
            an_by_ci = {}
            for ci in range(NCHUNK):
                if ci < NCHUNK - 1:
                    fillers.extend(proj_thunks(ci + 1))
                if ci >= 1:
                    fillers.extend(oproj_thunks(ci - 1, an_by_ci[ci - 1]))
                an_by_ci[ci] = attn_chunk(ci)
                drain_all()
            for fn, _ in oproj_thunks(NCHUNK - 1, an_by_ci[NCHUNK - 1]):
                fn()
    nc.compile()
    return nc


def _prep_in_maps(x, Wq, Wk, Wv, Wo):
    import jax.numpy as jnp

    def to_bf16(a):
        return np.asarray(jnp.asarray(np.asarray(a), dtype=jnp.bfloat16))

    i = np.arange(128)[:, None]
    j = np.arange(128)[None, :]
    msk = (i <= j).astype(np.float32)

    in_maps = []
    for c in range(N_CORES):
        b, g = c // 4, c % 4
        qh = [8 * g + a for a in range(8)]
        wq_cols = []
        for a in range(4):
            wq_cols.append(np.arange(qh[a] * HD, (qh[a] + 1) * HD))
            wq_cols.append(np.arange(qh[a + 4] * HD, (qh[a + 4] + 1) * HD))
        wq_r = np.asarray(Wq)[:, np.concatenate(wq_cols)]
        wo_r = np.asarray(Wo)[np.concatenate(wq_cols), :]
        wk_s = np.asarray(Wk)[:, 2 * g * HD: (2 * g + 2) * HD]
        wv_s = np.asarray(Wv)[:, 2 * g * HD: (2 * g + 2) * HD]
        in_maps.append({
            "xT": to_bf16(np.asarray(x)[b].T),
            "wq": to_bf16(wq_r),
            "wk": to_bf16(wk_s),
            "wv": to_bf16(wv_s),
            "wo": to_bf16(wo_r),
            "msk": to_bf16(msk),
        })
    return in_maps


def kernel(x, Wq, Wk, Wv, Wo, trace=False):
    if "nc" not in _CACHE:
        _CACHE["nc"] = _build()
    nc = _CACHE["nc"]
    in_maps = _prep_in_maps(x, Wq, Wk, Wv, Wo)
    res = bass_utils.run_bass_kernel_spmd(
        nc, in_maps, core_ids=list(range(N_CORES)), trace=trace)
    _CACHE["last_result"] = res
    out = np.zeros((B, N, D), np.float32)
    for c in range(N_CORES):
        out[c // 4] += np.asarray(res.results[c]["part"], dtype=np.float32)
    return out


# revision 17
# speedup vs baseline: 1.2569x; 1.1007x over previous
"""GroupedQueryAttention forward on 8 Trainium2 NeuronCores (Bass/Tile), v2.

Sharding (per spec hint): data-parallel over batch (B=2) x tensor-parallel
over KV-head groups (4 groups of 2 KV heads + their 8 query heads each).
Core c -> (batch b = c // 4, group g = c % 4).

v2 redesign vs baseline (573us):
  - scores row-tiled 2x: the two K=64 (per-kv-head) score matmuls run
    concurrently in disjoint PE row groups (tile_position (0,0)/(64,0)).
  - attn@V col-tiled 2x: the two M=64 AV matmuls run concurrently in
    disjoint PE col groups ((0,0)/(0,64)), both kv heads in one PSUM bank.
  - softmax denominators via 4 concurrent M=1 matmuls (ones stationary)
    col-tiled into one PSUM bank; reciprocal via DVE approx; broadcast to
    128 partitions via stride-0-source DMA; normalize fused into the PSUM
    evacuation tensor_tensor.
  - causal triangle trimming: diagonal-block matmuls/exps only cover the
    columns right of the diagonal (query >= key block start).
  - exp batched to FD=2*F per ACTIVATE from bf16 score PSUM banks.
  - mt-outer loop reuses kT/V stationaries across the 2 head pairs of a
    group; V transposed via DMA xbar instead of the PE.
  - software-pipelined emission: projection and o_proj matmul groups are
    interleaved into the attention loop as PE fillers so the tensor engine
    never idles while the scalar engine chews exps (keeps HAM at K=8/8).
  - o_proj partials written as bf16 (halves output DMA); host sums fp32.

All device compute bf16 with fp32 PSUM accumulation (bf16 PSUM for raw
scores only, pre-exp). Host pre-casts/pre-transposes x and pre-slices and
pair-reorders the weight shards.
"""

from collections import deque

import numpy as np

import concourse.bass as bass  # noqa: F401  (import keeps engine registry warm)
import concourse.mybir as mybir
import concourse.tile as tile
from concourse import bacc, bass_utils

# Problem shape (hardcoded per contract).
B, N, D = 2, 2048, 2048
NUM_HEADS = 32
NUM_KV_HEADS = 8
HD = 64
G = NUM_HEADS // NUM_KV_HEADS  # 4
N_CORES = 8
NT = D // 128                  # 16 contraction tiles
NCHUNK = 4                     # token chunks of 512
CH = 512

_CACHE = {}


def _build():
    nc = bacc.Bacc("TRN2", target_bir_lowering=False, debug=False,
                   num_devices=N_CORES)
    f32, bf16 = mybir.dt.float32, mybir.dt.bfloat16
    Exp = mybir.ActivationFunctionType.Exp

    xT = nc.dram_tensor("xT", [D, N], bf16, kind="ExternalInput")
    wq = nc.dram_tensor("wq", [D, 512], bf16, kind="ExternalInput")
    wk = nc.dram_tensor("wk", [D, 128], bf16, kind="ExternalInput")
    wv = nc.dram_tensor("wv", [D, 128], bf16, kind="ExternalInput")
    wo = nc.dram_tensor("wo", [512, D], bf16, kind="ExternalInput")
    msk = nc.dram_tensor("msk", [128, 128], bf16, kind="ExternalInput")
    part = nc.dram_tensor("part", [N, D], bf16, kind="ExternalOutput")

    with tile.TileContext(nc) as tc:
        with (
            tc.tile_pool(name="const", bufs=1) as cpool,
            tc.tile_pool(name="proj", bufs=1) as kpool,
            tc.tile_pool(name="work", bufs=2) as wpool,
            tc.tile_pool(name="ps_s", bufs=1, space="PSUM") as ps_s,
            tc.tile_pool(name="ps_pa", bufs=1, space="PSUM") as ps_pa,
            tc.tile_pool(name="ps_d", bufs=1, space="PSUM") as ps_d,
            tc.tile_pool(name="ps_proj", bufs=1, space="PSUM") as ps_proj,
        ):
            # ---- inputs / constants -------------------------------------
            wk_t = cpool.tile([128, NT * 128], bf16, tag="wk")
            nc.sync.dma_start(
                wk_t[:].rearrange("p (t o) -> p t o", t=NT),
                wk.ap().rearrange("(t p) o -> p t o", p=128))
            wv_t = cpool.tile([128, NT * 128], bf16, tag="wv")
            nc.sync.dma_start(
                wv_t[:].rearrange("p (t o) -> p t o", t=NT),
                wv.ap().rearrange("(t p) o -> p t o", p=128))
            # wq/wo/msk ride the scalar-issued DMA queue so they load in
            # parallel with the sync-queue xt chunks.
            wq_t = cpool.tile([128, NT * 512], bf16, tag="wq")
            nc.scalar.dma_start(
                wq_t[:].rearrange("p (t o) -> p t o", t=NT),
                wq.ap().rearrange("(t p) o -> p t o", p=128))
            wo_t = cpool.tile([128, 4 * D], bf16, tag="wo")
            nc.scalar.dma_start(
                wo_t[:].rearrange("p (t o) -> p t o", t=4),
                wo.ap().rearrange("(t p) o -> p t o", p=128))
            msk_t = cpool.tile([128, 128], bf16, tag="msk")
            nc.scalar.dma_start(msk_t[:], msk.ap()[:])
            xt = []
            for j in range(NCHUNK):
                xj = cpool.tile([128, NT * CH], bf16, tag=f"xt{j}")
                nc.sync.dma_start(
                    xj[:].rearrange("p (t n) -> p t n", t=NT),
                    xT.ap().rearrange("(t p) n -> p t n", p=128)
                    [:, :, j * CH:(j + 1) * CH])
                xt.append(xj)
            ones1 = cpool.tile([128, 1], bf16, tag="ones1")
            nc.vector.memset(ones1[:], 1.0)
            # pre-warm the exp activation table while the DMAs run
            scr = cpool.tile([1, 8], f32, tag="scr")
            nc.vector.memset(scr[:], 0.0)
            nc.scalar.activation(scr[0:1, :], scr[0:1, :], Exp)

            # persistent projection outputs
            kt = [kpool.tile([128, CH], bf16, tag=f"kt{j}", name=f"kt{j}")
                  for j in range(NCHUNK)]
            v3 = [kpool.tile([128, 128], bf16, tag=f"v3_{m}", name=f"v3_{m}")
                  for m in range(N // 128)]
            qt = [[kpool.tile([128, CH], bf16, tag=f"qt{a}_{j}",
                              name=f"qt{a}_{j}")
                   for j in range(NCHUNK)] for a in range(4)]

            # ---- filler machinery ---------------------------------------
            fillers = deque()  # (emit_fn, est_pe_ns)

            def drain(budget_ns):
                spent = 0
                while fillers and spent < budget_ns:
                    fn, est = fillers.popleft()
                    fn()
                    spent += est

            def drain_all():
                while fillers:
                    fn, _ = fillers.popleft()
                    fn()

            # ---- projection thunks --------------------------------------
            def proj_chain(dst_evac, lhsT_col, j, n_chunks=4):
                """Returns thunk fns for one 16-deep contraction chain.

                lhsT_col(t) -> AP for the stationary tile;
                dst_evac(ps) emits the evacuation."""
                ps_box = {}

                def quarter(q):
                    def emit():
                        if q == 0:
                            ps_box["ps"] = ps_proj.tile(
                                [128, CH], f32, tag="proj", name="pps")
                        ps = ps_box["ps"]
                        for t in range(4 * q, 4 * q + 4):
                            nc.tensor.matmul(
                                ps[:], lhsT_col(t),
                                xt[j][:, t * CH:(t + 1) * CH],
                                start=(t == 0), stop=(t == NT - 1))
                        if q == 3:
                            dst_evac(ps)
                    return emit
                return [(quarter(q), 1400) for q in range(4)]

            def proj_thunks(j):
                th = []

                def kev(ps):
                    nc.vector.tensor_copy(kt[j][:], ps[:])
                th += proj_chain(kev, lambda t: wk_t[:, t * 128:(t + 1) * 128], j)

                def vev(ps):
                    vt_s = wpool.tile([128, CH], bf16, tag="vt", name="vt_s")
                    nc.vector.tensor_copy(vt_s[:], ps[:])
                    for s in range(4):
                        nc.sync.dma_start_transpose(
                            v3[4 * j + s][:], vt_s[:, s * 128:(s + 1) * 128])
                th += proj_chain(vev, lambda t: wv_t[:, t * 128:(t + 1) * 128], j)
                for a in range(4):
                    def qev(ps, a=a):
                        nc.vector.tensor_copy(qt[a][j][:], ps[:])
                    th += proj_chain(
                        qev,
                        lambda t, a=a: wq_t[:, t * 512 + a * 128:
                                            t * 512 + (a + 1) * 128], j)
                return th

            # ---- o_proj thunks ------------------------------------------
            def oproj_thunks(ci, an_tiles, tail=False):
                n0 = ci * CH
                th = []
                for nt_ in range(4):
                    for dc in range(4):
                        def emit(nt_=nt_, dc=dc):
                            # in the tail (post-attention) phase the score
                            # banks are free: rotate over them too so the
                            # PE never waits on a single bank's evacuation
                            if tail and (nt_ * 4 + dc) % 2:
                                po = ps_s.tile([128, 1024], f32,
                                               tag=f"s{(nt_ * 4 + dc) // 2 % 2}",
                                               name="po2")[:, 0:CH]
                            else:
                                po = ps_proj.tile([128, CH], f32, tag="proj",
                                                  name="po")
                            for a in range(4):
                                nc.tensor.matmul(
                                    po[:],
                                    an_tiles[a][:, nt_ * 128:(nt_ + 1) * 128],
                                    wo_t[:, a * D + dc * CH:
                                         a * D + (dc + 1) * CH],
                                    start=(a == 0), stop=(a == 3))
                            st = wpool.tile([128, CH], bf16, tag="st",
                                            bufs=3, name="st")
                            if (nt_ + dc) % 2 == 0:
                                nc.scalar.copy(st[:], po[:])
                            else:
                                nc.vector.tensor_copy(st[:], po[:])
                            nc.gpsimd.dma_start(
                                part.ap()[n0 + nt_ * 128:n0 + (nt_ + 1) * 128,
                                          dc * CH:(dc + 1) * CH],
                                st[:])
                        th.append((emit, 1400))
                return th

            # ---- attention ----------------------------------------------
            def attn_chunk(ci):
                n0 = ci * CH
                M = 4 * ci + 4
                an_tiles = []
                for g in range(2):
                    a0, a1 = 2 * g, 2 * g + 1
                    paA = ps_pa.tile([128, CH], f32, tag="paA", name="paA")
                    paB = ps_pa.tile([128, CH], f32, tag="paB", name="paB")
                    dn = ps_d.tile([128, CH], f32, tag="dn", name="dn")
                    pts = {}

                    def scores_exp(mt):
                        # score layout per head pair a: kv0 block at cols
                        # [0:F], kv1 block at cols [512:512+F] (bank 2);
                        # one FD=512+F exp covers both (cols [F:512] are
                        # junk for diagonal tiles and never read).
                        jmt, cmt = mt // 4, mt % 4
                        flo = max(0, (mt - 4 * ci) * 128)
                        F = CH - flo
                        # kv1 block always in bank 1: the two row-tiled
                        # score matmuls run concurrently and must not
                        # target the same PSUM bank (cols [F:CH] junk)
                        off = CH
                        pt_pair = []
                        for i, a in enumerate((a0, a1)):
                            ss = ps_s.tile([128, 1024], f32, tag=f"s{i}",
                                           name="ss")
                            nc.tensor.matmul(
                                ss[:, 0:F],
                                kt[jmt][0:64, cmt * 128:(cmt + 1) * 128],
                                qt[a][ci][0:64, flo:CH],
                                start=True, stop=True, tile_position=(0, 0))
                            nc.tensor.matmul(
                                ss[:, off:off + F],
                                kt[jmt][64:128, cmt * 128:(cmt + 1) * 128],
                                qt[a][ci][64:128, flo:CH],
                                start=True, stop=True, tile_position=(64, 0))
                            pt_ = wpool.tile([128, 1024], bf16, tag=f"pt{i}",
                                             name="pt")
                            nc.scalar.activation(pt_[:, 0:off + F],
                                                 ss[:, 0:off + F],
                                                 Exp, scale=0.125)
                            if mt >= 4 * ci:  # diagonal block: causal mask
                                nc.vector.tensor_mul(
                                    pt_[:, 0:128], pt_[:, 0:128], msk_t[:])
                                nc.vector.tensor_mul(
                                    pt_[:, off:off + 128],
                                    pt_[:, off:off + 128], msk_t[:])
                            pt_pair.append(pt_)
                        pts[mt] = (pt_pair, flo, F, off)

                    def av_denom(mt):
                        pt_pair, flo, F, off = pts.pop(mt)
                        first, last = (mt == 0), (mt == M - 1)
                        for i in range(2):
                            pa = paA if i == 0 else paB
                            pt_ = pt_pair[i]
                            nc.tensor.matmul(
                                pa[0:64, flo:CH], v3[mt][:, 0:64],
                                pt_[:, 0:F],
                                start=first, stop=last, tile_position=(0, 0))
                            nc.tensor.matmul(
                                pa[64:128, flo:CH], v3[mt][:, 64:128],
                                pt_[:, off:off + F],
                                start=first, stop=last, tile_position=(0, 64))
                        for pos, src in ((0, pt_pair[0][:, 0:F]),
                                         (32, pt_pair[0][:, off:off + F]),
                                         (64, pt_pair[1][:, 0:F]),
                                         (96, pt_pair[1][:, off:off + F])):
                            nc.tensor.matmul(
                                dn[pos:pos + 1, flo:CH], ones1[:], src,
                                start=first, stop=last,
                                tile_position=(0, pos))

                    for mt in range(M):
                        scores_exp(mt)
                        if mt > 0:
                            av_denom(mt - 1)
                        flo = max(0, (mt - 4 * ci) * 128)
                        F = CH - flo
                        slack = 2 * (2 * F + 352) / 1.2 - (5 * F / 2.4 + 400)
                        drain(max(0, slack))
                    av_denom(M - 1)

                    # Evacuate pa raw (frees the accumulator banks for the
                    # next group immediately); normalize off-critical-path.
                    aots = []
                    for i in range(2):
                        aot = wpool.tile([128, CH], bf16, tag=f"aot{i}",
                                         name="aot")
                        nc.vector.tensor_copy(aot[:], (paA if i == 0 else paB)[:])
                        aots.append(aot)
                    d4r = wpool.tile([128, CH], f32, tag="d4r", name="d4r")
                    nc.vector.reciprocal_approx_fast(d4r[0:97, :], dn[0:97, :])
                    for i, a in enumerate((a0, a1)):
                        rbr = wpool.tile([128, CH], f32, tag=f"rbr{i}",
                                         name="rbr")
                        r0, r1 = 64 * i, 64 * i + 32
                        nc.gpsimd.dma_start(
                            rbr[0:64, :],
                            d4r[r0:r0 + 1, :].unsqueeze(1)
                            .broadcast_to([1, 64, CH]))
                        nc.gpsimd.dma_start(
                            rbr[64:128, :],
                            d4r[r1:r1 + 1, :].unsqueeze(1)
                            .broadcast_to([1, 64, CH]))
                        an = wpool.tile([128, CH], bf16, tag=f"an{a}",
                                        name=f"an{a}")
                        nc.vector.tensor_mul(an[:], aots[i][:], rbr[:])
                        an_tiles.append(an)
                return an_tiles

            # ---- main schedule ------------------------------------------
            for fn, _ in proj_thunks(0):
                fn()
            an_by_ci = {}
            for ci in range(NCHUNK):
                if ci < NCHUNK - 1:
                    fillers.extend(proj_thunks(ci + 1))
                if ci >= 1:
                    fillers.extend(oproj_thunks(ci - 1, an_by_ci[ci - 1]))
                an_by_ci[ci] = attn_chunk(ci)
                drain_all()
            for fn, _ in oproj_thunks(NCHUNK - 1, an_by_ci[NCHUNK - 1],
                                      tail=True):
                fn()
    nc.compile()
    return nc


def _prep_in_maps(x, Wq, Wk, Wv, Wo):
    import jax.numpy as jnp

    def to_bf16(a):
        return np.asarray(jnp.asarray(np.asarray(a), dtype=jnp.bfloat16))

    i = np.arange(128)[:, None]
    j = np.arange(128)[None, :]
    msk = (i <= j).astype(np.float32)

    in_maps = []
    for c in range(N_CORES):
        b, g = c // 4, c % 4
        qh = [8 * g + a for a in range(8)]
        wq_cols = []
        for a in range(4):
            wq_cols.append(np.arange(qh[a] * HD, (qh[a] + 1) * HD))
            wq_cols.append(np.arange(qh[a + 4] * HD, (qh[a + 4] + 1) * HD))
        wq_r = np.asarray(Wq)[:, np.concatenate(wq_cols)]
        wo_r = np.asarray(Wo)[np.concatenate(wq_cols), :]
        wk_s = np.asarray(Wk)[:, 2 * g * HD: (2 * g + 2) * HD]
        wv_s = np.asarray(Wv)[:, 2 * g * HD: (2 * g + 2) * HD]
        in_maps.append({
            "xT": to_bf16(np.asarray(x)[b].T),
            "wq": to_bf16(wq_r),
            "wk": to_bf16(wk_s),
            "wv": to_bf16(wv_s),
            "wo": to_bf16(wo_r),
            "msk": to_bf16(msk),
        })
    return in_maps


def kernel(x, Wq, Wk, Wv, Wo, trace=False):
    if "nc" not in _CACHE:
        _CACHE["nc"] = _build()
    nc = _CACHE["nc"]
    in_maps = _prep_in_maps(x, Wq, Wk, Wv, Wo)
    res = bass_utils.run_bass_kernel_spmd(
        nc, in_maps, core_ids=list(range(N_CORES)), trace=trace)
    _CACHE["last_result"] = res
    out = np.zeros((B, N, D), np.float32)
    for c in range(N_CORES):
        out[c // 4] += np.asarray(res.results[c]["part"], dtype=np.float32)
    return out
